# revision 82
# baseline (speedup 1.0000x reference)
"""Causal multi-head attention block (QKV proj -> causal MHA -> out proj) on 8 Trainium2
cores.

Sharding: core = b*2 + hh handles batch b (of 4) and head-half hh (8 of 16 heads),
computing attention for its heads over the full sequence, then a partial output
projection over its 512 y-channels for all 2048 tokens. A pairwise ReduceScatter
([0,1],[2,3],...) sums the two partials of each batch and leaves each core with its
token-half of the final output. Host-side work is pure slicing/concatenation.

The QKV projection (per 512-token tile) is interleaved with the attention of the
previous query tile so the PE fills the Act-bound softmax phase: emission alternates
attention inner-loop iterations with "pieces" (QKV matmul groups, x^T transposes,
out-proj chunks). Attention itself is software-pipelined: scores(kg+1) is emitted
before attnV(kg) so the exp chain on the Act engine never starves.

attnV computes y transposed (queries on PSUM partitions, via a ones column in V for
the softmax denominator) so normalization is a per-partition tensor_scalar multiply;
y is then transposed back on the PE for the output projection.

dtypes: bf16 throughout the matmul operands (x^T, Q^T/K^T/V, attention weights,
y, Wo — inputs converted host-side); all PSUM accumulation is f32 and the softmax
denominator is accumulated from the same bf16 weights, so normalization is exact
w.r.t. rounding. The V bias is folded into the output bias host-side (attn rows
sum to 1 after normalization).
"""

import numpy as np

import concourse.bass as bass
import concourse.tile as tile
from concourse import bacc, mybir
from concourse.bass_utils import run_bass_kernel_spmd

F32 = mybir.dt.float32
F32R = mybir.dt.float32r
BF16 = mybir.dt.bfloat16
AF = mybir.ActivationFunctionType

B, T, C, H = 4, 2048, 1024, 16
D = C // H          # 64
NHL = H // 2        # 8 local heads per core
NHP = NHL // 2      # 4 local head pairs
FL = NHL * D        # 512 local features
NCC = C // 128      # 8 contraction chunks over C
NTB = T // 128      # 16 token blocks
NTT = T // 512      # 4 token tiles / qtiles
NEG = -1.0e30


def build():
    nc = bacc.Bacc("TRN2", target_bir_lowering=False, num_devices=8)

    xb = nc.dram_tensor("xb", [T, C], BF16, kind="ExternalInput")
    wq = nc.dram_tensor("wq", [C, FL], BF16, kind="ExternalInput")
    wk = nc.dram_tensor("wk", [C, FL], BF16, kind="ExternalInput")
    wv = nc.dram_tensor("wv", [C, FL], BF16, kind="ExternalInput")
    wo = nc.dram_tensor("wo", [FL, C], BF16, kind="ExternalInput")
    bq = nc.dram_tensor("bq", [FL], F32, kind="ExternalInput")
    bk = nc.dram_tensor("bk", [FL], F32, kind="ExternalInput")
    bob = nc.dram_tensor("bob", [128, C], F32, kind="ExternalInput")  # (bo/2+bv@Wo) bcast
    ident = nc.dram_tensor("ident", [128, 128], F32R, kind="ExternalInput")
    identb = nc.dram_tensor("identb", [128, 128], BF16, kind="ExternalInput")
    mask4 = nc.dram_tensor("mask4", [128, 128], F32, kind="ExternalInput")
    zh = nc.dram_tensor("zh", [T // 2, C], F32, kind="ExternalOutput")

    with tile.TileContext(nc) as tc:
        with (
            tc.tile_pool(name="res", bufs=1) as res,
            tc.tile_pool(name="dram", bufs=1, space="DRAM") as dram,
        ):
            # resident: Q^T, K^T bf16 [128, 4hp x 2048tok]; V+ones bf16
            # [128, 16tb x 520]; mask, identity + out-proj consts.
            qt_sb = res.tile([128, NHP * T], BF16)
            kt_sb = res.tile([128, NHP * T], BF16)
            v_sb = res.tile([128, NTB * (NHL * 65)], BF16)
            m4_sb = res.tile([128, 128], F32, name="m4")
            wo_sb = res.tile([128, NHP * C], BF16, name="wo_sb")
            bob_sb = res.tile([128, C], F32, name="bob_sb")
            id_sb = res.tile([128, 128], F32R, name="id_sb")
            idb_sb = res.tile([128, 128], BF16, name="idb_sb")
            zpart = dram.tile([T, C], F32)
            zreds = [
                dram.tile([128, C], F32, name=f"zred{i}") for i in range(8)
            ]

            with (
                tc.tile_pool(name="p1c", bufs=1) as p1c,
                tc.tile_pool(name="p1", bufs=6) as p1,
                tc.tile_pool(name="xtp", bufs=1) as xtp,
                tc.tile_pool(name="ytp", bufs=4) as ytp,
                tc.tile_pool(name="ysb_pool", bufs=1) as ysb_pool,
                tc.tile_pool(name="p2", bufs=10) as p2,
                tc.tile_pool(name="norm", bufs=8) as norm,
                tc.tile_pool(name="p3", bufs=6) as p3,
                tc.tile_pool(name="s_ps", bufs=2, space="PSUM") as s_ps_pool,
                tc.tile_pool(name="yu_ps", bufs=2, space="PSUM") as yu_ps_pool,
                tc.tile_pool(name="scr_ps", bufs=2, space="PSUM") as scr_ps_pool,
            ):
                ysb = ysb_pool.tile([128, NHP * T], BF16)

                # ---- preamble DMAs (order = SP queue order) ----
                xnats = {}

                def load_xnat(tb):
                    xnats[tb] = p1.tile([128, C], BF16, tag="xnat",
                                        name=f"xnat{tb}")
                    nc.sync.dma_start(xnats[tb][:], xb[tb * 128:(tb + 1) * 128, :])

                load_xnat(0)
                nc.sync.dma_start(idb_sb[:], identb[:, :])
                nc.sync.dma_start(id_sb[:], ident[:, :])
                for tb in range(1, 4):
                    load_xnat(tb)
                # K/Q weights in per-fb chunks, in first-use order, so the
                # first K matmul group can start as soon as possible
                wk_sb = p1c.tile([128, NCC * FL], BF16, tag="wk")
                wq_sb = p1c.tile([128, NCC * FL], BF16, tag="wq")
                wv_sb = p1c.tile([128, NCC * FL], BF16, tag="wv")

                for w_sb, w_dram in ((wk_sb, wk), (wq_sb, wq)):
                    nc.sync.dma_start(
                        w_sb[:].rearrange("p (c f) -> p c f", c=NCC),
                        w_dram.rearrange("(c p) f -> p c f", p=128),
                    )
                bq_sb = p1c.tile([128, NHP], F32, tag="bq")
                nc.sync.dma_start(bq_sb[:], bq.rearrange("(f p) -> p f", p=128))
                bk_sb = p1c.tile([128, NHP], F32, tag="bk")
                nc.sync.dma_start(bk_sb[:], bk.rearrange("(f p) -> p f", p=128))
                # warm the exp table (hides ~2.7us ACT_TABLE_LOAD)
                warm = p1c.tile([1, 1], F32, tag="warm")
                nc.scalar.activation(warm[:], id_sb[0:1, 0:1].bitcast(F32), AF.Exp)
                for half in range(2):
                    nc.sync.dma_start(
                        wv_sb[:, half * 4 * FL:(half + 1) * 4 * FL].rearrange(
                            "p (c f) -> p c f", c=NCC // 2),
                        wv[half * 512:(half + 1) * 512, :].rearrange(
                            "(c p) f -> p c f", p=128),
                    )
                for tb in range(4, 8):
                    load_xnat(tb)
                nc.sync.dma_start(m4_sb[:], mask4[:, :])
                nc.sync.dma_start(
                    wo_sb[:].rearrange("p (c n) -> p c n", c=NHP),
                    wo.rearrange("(c p) n -> p c n", p=128),
                )
                nc.sync.dma_start(bob_sb[:], bob[:, :])

                # ---- phase-1 pieces: x^T transposes and QKV projections ----
                xts = {}

                def xt_alloc(tt):
                    xts[tt] = xtp.tile([128, NCC * 512], BF16, tag="xt",
                                       name=f"xt{tt}")

                def tp_piece(tt, k):
                    # transpose token block tb = 4*tt+k into x^T tile tt
                    # (blocks 0-7 are queued in the preamble)
                    tb = 4 * tt + k
                    if tb + 4 < NTB and tb + 4 >= 8:
                        load_xnat(tb + 4)
                    xnat = xnats.pop(tb)
                    xt = xts[tt]
                    for cg in range(NCC // 4):
                        tp_ps = scr_ps_pool.tile([128, 512], BF16, tag="scr",
                                                 name=f"tp{tb}_{cg}")
                        for kk in range(4):
                            cc = cg * 4 + kk
                            nc.tensor.transpose(
                                tp_ps[:, kk * 128:(kk + 1) * 128],
                                xnat[:, cc * 128:(cc + 1) * 128], idb_sb[:]
                            )
                        # alternate DVE/Act so the copy chain isn't serialized
                        dst = xt[:].rearrange("p (c t) -> p c t", c=NCC)[
                            :, cg * 4:(cg + 1) * 4, k * 128:(k + 1) * 128
                        ]
                        src = tp_ps[:].rearrange("p (k t) -> p k t", k=4)
                        nc.vector.tensor_scalar_add(dst, src, 0.0)

                def qk_piece(tt, fb, w_sb, b_sb, dst):
                    xt = xts[tt]
                    ps = scr_ps_pool.tile([128, 512], F32, tag="scr",
                                          name=f"ps{tt}_{fb}")
                    for cc in range(NCC):
                        nc.tensor.matmul(
                            ps[:],
                            w_sb[:, cc * FL + fb * 128: cc * FL + (fb + 1) * 128],
                            xt[:, cc * 512:(cc + 1) * 512],
                            start=(cc == 0),
                            stop=(cc == NCC - 1),
                        )
                    nc.vector.tensor_scalar_add(
                        dst[:, fb * T + tt * 512: fb * T + (tt + 1) * 512],
                        ps[:],
                        b_sb[:, fb:fb + 1],
                    )

                def v_piece(tt, k):
                    tb = 4 * tt + k
                    xt = xts[tt]
                    ps = scr_ps_pool.tile([128, 512], F32, tag="scr",
                                          name=f"psv{tb}")
                    for cc in range(NCC):
                        nc.tensor.matmul(
                            ps[:],
                            xt[:, cc * 512 + k * 128: cc * 512 + (k + 1) * 128],
                            wv_sb[:, cc * FL:(cc + 1) * FL],
                            start=(cc == 0),
                            stop=(cc == NCC - 1),
                        )
                    vslice = v_sb[:, tb * (NHL * 65):(tb + 1) * (NHL * 65)]
                    v3 = vslice.rearrange("p (h c) -> p h c", h=NHL)
                    nc.vector.tensor_scalar_add(
                        v3[:, :, 0:D],
                        ps[:].rearrange("p (h d) -> p h d", h=NHL),
                        0.0,
                    )
                    nc.gpsimd.memset(v3[:, :, D:D + 1], 1.0)

                def qkv_pieces(tt):
                    return (
                        [lambda fb=fb: qk_piece(tt, fb, wk_sb, bk_sb, kt_sb)
                         for fb in range(NHP)]
                        + [lambda fb=fb: qk_piece(tt, fb, wq_sb, bq_sb, qt_sb)
                           for fb in range(NHP)]
                        + [lambda k=k: v_piece(tt, k) for k in range(4)]
                    )

                def tp_pieces(tt):
                    def first(k):
                        def go():
                            if k == 0:
                                xt_alloc(tt)
                            tp_piece(tt, k)
                        return go
                    return [first(k) for k in range(4)]

                # ---- out-proj + ReduceScatter pieces ----
                # zpart rows chunk-major: chunk c holds tb c then tb 8+c, so
                # each pairwise ReduceScatter chunk is one contiguous block.
                ZROW = {}
                for c in range(8):
                    ZROW[c] = c * 256
                    ZROW[8 + c] = c * 256 + 128

                def op_chunk(tb, ct):
                    zrow = ZROW[tb]
                    zps = scr_ps_pool.tile([128, 512], F32, tag="scr",
                                           name=f"z{tb}_{ct}")
                    for cc in range(NHP):
                        nc.tensor.matmul(
                            zps[:],
                            ysb[:, cc * T + tb * 128: cc * T + (tb + 1) * 128],
                            wo_sb[:, cc * C + ct * 512: cc * C + (ct + 1) * 512],
                            start=(cc == 0),
                            stop=(cc == NHP - 1),
                        )
                    z_sb = p3.tile(
                        [128, 512], F32, tag="zsb", name=f"zsb{tb}_{ct}"
                    )
                    nc.vector.tensor_add(
                        z_sb[:], zps[:], bob_sb[:, ct * 512:(ct + 1) * 512]
                    )
                    nc.sync.dma_start(
                        zpart[zrow:zrow + 128, ct * 512:(ct + 1) * 512],
                        z_sb[:],
                    )

                def rs_chunk(c):
                    nc.gpsimd.collective_compute(
                        "ReduceScatter",
                        mybir.AluOpType.add,
                        replica_groups=[[0, 1], [2, 3], [4, 5], [6, 7]],
                        ins=[zpart[c * 256:(c + 1) * 256, :].opt()],
                        outs=[zreds[c].opt()],
                    )
                    nc.sync.dma_start(
                        zh[c * 128:(c + 1) * 128, :], zreds[c][:]
                    )

                def op_item(tb, ct, c=None):
                    def go():
                        op_chunk(tb, ct)
                        if c is not None:
                            rs_chunk(c)
                    return go

                # ---- piece queue: dripped into attention units ----
                queue = []
                emitted = [0]

                def drip():
                    if queue:
                        emitted[0] += 1
                        queue.pop(0)()

                def drain(n_left=0):
                    while len(queue) > n_left:
                        drip()

                # ---- attention for one (qtile, head-pair) unit ----
                y_ts = {}

                def attention_unit(qt, hp):
                    if qt not in y_ts:
                        # y_t: qtile's y, token-major [128 tok, 4tb x 512f]
                        y_ts[qt] = ytp.tile([128, 4 * 512], BF16, tag="yt",
                                            name=f"yt{qt}")
                    y_t = y_ts[qt]
                    if True:
                        n_kb = 4 * (qt + 1)
                        n_kg = n_kb // 2
                        # yu_t: attention out transposed, [128 q, 4qoff x
                        # (64d + rowsum)] per hi; queries on partitions so the
                        # softmax denominator is a per-partition scalar.
                        yus = [
                            yu_ps_pool.tile([128, 4 * 65], F32, tag="yu",
                                            name=f"yu{qt}_{hp}_{i}")
                            for i in range(2)
                        ]
                        qsl = qt_sb[:, hp * T + qt * 512: hp * T + (qt + 1) * 512]
                        sss = {}

                        def emit_scores(kg, hi=None):
                            if hi is None:
                                sss[kg] = [
                                    s_ps_pool.tile([128, 1024], F32, tag="s",
                                                   name=f"s{qt}_{hp}_{kg}_{i}")
                                    for i in range(2)
                                ]
                                for h2 in range(2):
                                    emit_scores(kg, h2)
                                return
                            ss = sss[kg]
                            for c2 in range(2):
                                kb = kg * 2 + c2
                                c = kb - 4 * qt
                                # c==1 writes the full block so the
                                # untrimmed exp never reads unwritten
                                # PSUM (the extra cols are discarded)
                                j0 = c * 128 if c >= 2 else 0
                                nc.tensor.matmul(
                                    ss[hi][:, c2 * 512 + j0:(c2 + 1) * 512],
                                    kt_sb[
                                        hi * 64:(hi + 1) * 64,
                                        hp * T + kb * 128:
                                        hp * T + (kb + 1) * 128,
                                    ],
                                    qsl[hi * 64:(hi + 1) * 64, j0:],
                                    tile_position=(hi * 64, 0),
                                    start=True,
                                    stop=True,
                                )
                            for c2 in range(2):
                                c = kg * 2 + c2 - 4 * qt
                                if 0 <= c <= 3:
                                    # triangular band [c*128,(c+1)*128):
                                    # same i>j triangle for every c
                                    b0 = c2 * 512 + c * 128
                                    nc.vector.tensor_add(
                                        ss[hi][:, b0:b0 + 128],
                                        ss[hi][:, b0:b0 + 128],
                                        m4_sb[:, 0:128],
                                    )

                        # software pipeline: scores(kg+1) emitted BEFORE
                        # attnV(kg) so the exp chain never starves the Act.
                        emit_scores(0)
                        for kg in range(n_kg):
                            ss = sss.pop(kg)
                            j0r = []
                            for c2 in range(2):
                                c = kg * 2 + c2 - 4 * qt
                                j0r.append(c * 128 if c > 0 else 0)
                            # per-hi: exp -> scores(kg+1,hi) -> attnV(kg,hi)
                            # so the PE serves work in readiness order (the
                            # hi1 scores are only legal after exp(kg,hi1) and
                            # must not block the already-ready attnV(kg,hi0))
                            for hi in range(2):
                                at = p2.tile([128, 1024], BF16, tag="attn")
                                if j0r[0] >= 256:
                                    # heavily masked pair: exp valid suffixes
                                    nc.scalar.activation(
                                        at[:, j0r[0]:512], ss[hi][:, j0r[0]:512],
                                        AF.Exp, scale=0.125,
                                    )
                                    nc.scalar.activation(
                                        at[:, 512 + j0r[1]:1024],
                                        ss[hi][:, 512 + j0r[1]:1024],
                                        AF.Exp, scale=0.125,
                                    )
                                else:
                                    nc.scalar.activation(
                                        at[:], ss[hi][:], AF.Exp, scale=0.125
                                    )
                                if kg + 1 < n_kg:
                                    if hi == 0:
                                        sss[kg + 1] = [
                                            s_ps_pool.tile(
                                                [128, 1024], F32, tag="s",
                                                name=f"s{qt}_{hp}_{kg + 1}_{i}")
                                            for i in range(2)
                                        ]
                                    emit_scores(kg + 1, hi)
                                h = 2 * hp + hi
                                for c2 in range(2):
                                    kb = kg * 2 + c2
                                    vsl = v_sb[
                                        :,
                                        kb * (NHL * 65) + h * 65:
                                        kb * (NHL * 65) + h * 65 + 65,
                                    ]
                                    # one accumulation group per yu PSUM bank:
                                    # start=True clears the whole 2KB zero
                                    # region, so only the very first write may
                                    # set it; later first-touch columns are
                                    # overwritten via per-element has_written.
                                    for qoff in range(max(kb - 4 * qt, 0), 4):
                                        nc.tensor.matmul(
                                            yus[hi][:, qoff * 65:
                                                   (qoff + 1) * 65],
                                            at[:, c2 * 512 + qoff * 128:
                                               c2 * 512 + (qoff + 1) * 128],
                                            vsl,
                                            start=(kb == 0 and qoff == 0),
                                            stop=(kb == n_kb - 1),
                                        )
                                if hi == 0:
                                    drip()
                        # normalize: y = y_u * (1/rowsum); rowsum is col 64 of
                        # each 65-block = a per-partition scalar. No broadcast.
                        for hi in range(2):
                            yu3 = yus[hi][:].rearrange("p (b c) -> p b c", b=4)
                            rcp = norm.tile([128, 4], F32, tag="rcp")
                            nc.vector.reciprocal(
                                rcp[:].unsqueeze(2), yu3[:, :, 64:65]
                            )
                            h = 2 * hp + hi
                            for qoff in range(4):
                                nc.vector.tensor_scalar_mul(
                                    y_t[:, qoff * 512 + h * 64:
                                        qoff * 512 + h * 64 + 64],
                                    yus[hi][:, qoff * 65: qoff * 65 + 64],
                                    rcp[:, qoff:qoff + 1],
                                )
                def finish_qt(qt):
                    # transpose y_t -> ysb feature-major for the out-proj
                    y_t = y_ts[qt]
                    for tl in range(4):
                        tb = 4 * qt + tl
                        tp = scr_ps_pool.tile([128, 512], BF16, tag="scr",
                                              name=f"ytp{qt}_{tl}")
                        for fc in range(NHP):
                            nc.tensor.transpose(
                                tp[:, fc * 128:(fc + 1) * 128],
                                y_t[:, tl * 512 + fc * 128:
                                    tl * 512 + (fc + 1) * 128],
                                idb_sb[:],
                            )
                        nc.vector.tensor_scalar_add(
                            ysb[:].rearrange("p (h t) -> p h t", h=NHP)[
                                :, :, tb * 128:(tb + 1) * 128],
                            tp[:].rearrange("p (h t) -> p h t", h=NHP),
                            0.0,
                        )
                    # hold all out-proj chunks for the last qtile's
                    # window, where the PE otherwise starves (Act-bound)
                    if qt == 2:
                        queue.extend(op_item(tb, ct)
                                     for tb in range(8) for ct in range(2))
                        queue.extend(op_item(8 + c, ct, c if ct else None)
                                     for c in range(4) for ct in range(2))

                # ---- schedule ----
                # prologue: QKV of tile 0 and transposes of tiles 0-1 run
                # before any attention (PE-heavy, Act idle).
                for piece in tp_pieces(0):
                    piece()
                for piece in qkv_pieces(0):
                    piece()
                for piece in tp_pieces(1):
                    piece()

                # queue pieces in dependency order, recording for each
                # attention unit (qt,hp) how many pieces must be emitted
                # before it (its K/Q columns and all V of its key range).
                prereq = {(0, hp): 0 for hp in range(NHP)}
                total = [0]

                def app(ps):
                    queue.extend(ps)
                    total[0] += len(ps)

                def app_qkv(tt):
                    qp = qkv_pieces(tt)  # [K fb0..3, Q fb0..3, V k0..3]
                    app([qp[0], qp[4]])          # K fb0, Q fb0
                    app(qp[8:12])                # V k0..3
                    prereq[(tt, 0)] = total[0]
                    for fb in range(1, NHP):
                        app([qp[fb], qp[4 + fb]])
                        prereq[(tt, fb)] = total[0]

                app_qkv(1)
                app(tp_pieces(2))
                app_qkv(2)
                app(tp_pieces(3))
                app_qkv(3)

                unit_seq = [(0, 0), (0, 1), (0, 2), (0, 3),
                            (1, 0), (1, 1), (1, 2), (1, 3),
                            (2, 0), (2, 1), (2, 2), (2, 3),
                            (3, 0), (3, 1), (3, 2), (3, 3)]
                left = {qt: NHP for qt in range(NTT)}
                for qt, hp in unit_seq:
                    while emitted[0] < prereq[(qt, hp)] and queue:
                        drip()
                    attention_unit(qt, hp)
                    left[qt] -= 1
                    if left[qt] == 0:
                        finish_qt(qt)
                drain()
                for i in range(4):
                    op_chunk(12 + i, 0)
                    op_chunk(12 + i, 1)
                    rs_chunk(4 + i)

    nc.compile()
    return nc


_NC_CACHE = None


def _get_nc():
    global _NC_CACHE
    if _NC_CACHE is None:
        _NC_CACHE = build()
    return _NC_CACHE


def _in_maps(x, Wqkv, bqkv, Wo, bo):
    x = np.ascontiguousarray(np.asarray(x, dtype=np.float32))
    Wqkv = np.ascontiguousarray(np.asarray(Wqkv, dtype=np.float32))
    bqkv = np.asarray(bqkv, dtype=np.float32)
    Wo = np.ascontiguousarray(np.asarray(Wo, dtype=np.float32))
    bo = np.asarray(bo, dtype=np.float32)

    ident = np.eye(128, dtype=np.float32)
    i_ = np.arange(128, dtype=np.int64)[:, None]
    j_ = np.arange(128, dtype=np.int64)[None, :]
    mask4 = np.where(i_ > j_, np.float32(NEG), np.float32(0.0)).astype(np.float32)

    from ml_dtypes import bfloat16
    identb = np.eye(128, dtype=bfloat16)
    in_maps = []
    for core in range(8):
        b, hh = core // 2, core % 2
        sl = slice(hh * FL, (hh + 1) * FL)
        bv_loc = bqkv[2 * C:][sl]
        wo_loc = np.ascontiguousarray(Wo[sl, :])
        # V bias folded into output bias: attn rows sum to 1 after normalize
        bo_loc = bo * 0.5 + bv_loc @ wo_loc
        in_maps.append({
            "xb": x[b].astype(bfloat16),
            "wq": np.ascontiguousarray(Wqkv[:, 0 * C:1 * C][:, sl]).astype(bfloat16),
            "wk": np.ascontiguousarray(Wqkv[:, 1 * C:2 * C][:, sl]).astype(bfloat16),
            "wv": np.ascontiguousarray(Wqkv[:, 2 * C:3 * C][:, sl]).astype(bfloat16),
            "wo": wo_loc.astype(bfloat16),
            "bq": np.ascontiguousarray(bqkv[0 * C:1 * C][sl]),
            "bk": np.ascontiguousarray(bqkv[1 * C:2 * C][sl]),
            "bob": np.broadcast_to(bo_loc[None, :], (128, C)).copy(),
            "ident": ident,
            "identb": identb,
            "mask4": mask4,
        })

    return in_maps


def _assemble(res):
    out = np.empty((B, T, C), dtype=np.float32)
    for b in range(B):
        out[b, : T // 2] = res.results[2 * b]["zh"]
        out[b, T // 2:] = res.results[2 * b + 1]["zh"]
    return out


def kernel(x, Wqkv, bqkv, Wo, bo):
    in_maps = _in_maps(x, Wqkv, bqkv, Wo, bo)
    res = run_bass_kernel_spmd(_get_nc(), in_maps, core_ids=list(range(8)))
    return _assemble(res)


def run_traced(x, Wqkv, bqkv, Wo, bo, trace_cores=None):
    in_maps = _in_maps(x, Wqkv, bqkv, Wo, bo)
    res = run_bass_kernel_spmd(
        _get_nc(), in_maps, core_ids=list(range(8)), trace=True,
        trace_cores=trace_cores,
    )
    return res


# revision 83
# speedup vs baseline: 1.0058x; 1.0058x over previous
"""Causal multi-head attention block (QKV proj -> causal MHA -> out proj) on 8 Trainium2
cores.

Sharding: core = b*2 + hh handles batch b (of 4) and head-half hh (8 of 16 heads),
computing attention for its heads over the full sequence, then a partial output
projection over its 512 y-channels for all 2048 tokens. A pairwise ReduceScatter
([0,1],[2,3],...) sums the two partials of each batch and leaves each core with its
token-half of the final output. Host-side work is pure slicing/concatenation.

The QKV projection (per 512-token tile) is interleaved with the attention of the
previous query tile so the PE fills the Act-bound softmax phase: emission alternates
attention inner-loop iterations with "pieces" (QKV matmul groups, x^T transposes,
out-proj chunks). Attention itself is software-pipelined: scores(kg+1) is emitted
before attnV(kg) so the exp chain on the Act engine never starves.

attnV computes y transposed (queries on PSUM partitions, via a ones column in V for
the softmax denominator) so normalization is a per-partition tensor_scalar multiply;
y is then transposed back on the PE for the output projection.

dtypes: bf16 throughout the matmul operands (x^T, Q^T/K^T/V, attention weights,
y, Wo — inputs converted host-side); all PSUM accumulation is f32 and the softmax
denominator is accumulated from the same bf16 weights, so normalization is exact
w.r.t. rounding. The V bias is folded into the output bias host-side (attn rows
sum to 1 after normalization).
"""

import numpy as np

import concourse.bass as bass
import concourse.tile as tile
from concourse import bacc, mybir
from concourse.bass_utils import run_bass_kernel_spmd

F32 = mybir.dt.float32
F32R = mybir.dt.float32r
BF16 = mybir.dt.bfloat16
AF = mybir.ActivationFunctionType

B, T, C, H = 4, 2048, 1024, 16
D = C // H          # 64
NHL = H // 2        # 8 local heads per core
NHP = NHL // 2      # 4 local head pairs
FL = NHL * D        # 512 local features
NCC = C // 128      # 8 contraction chunks over C
NTB = T // 128      # 16 token blocks
NTT = T // 512      # 4 token tiles / qtiles
NEG = -1.0e30


def build():
    nc = bacc.Bacc("TRN2", target_bir_lowering=False, num_devices=8)

    xb = nc.dram_tensor("xb", [T, C], BF16, kind="ExternalInput")
    wq = nc.dram_tensor("wq", [C, FL], BF16, kind="ExternalInput")
    wk = nc.dram_tensor("wk", [C, FL], BF16, kind="ExternalInput")
    wv = nc.dram_tensor("wv", [C, FL], BF16, kind="ExternalInput")
    wo = nc.dram_tensor("wo", [FL, C], BF16, kind="ExternalInput")
    bq = nc.dram_tensor("bq", [FL], F32, kind="ExternalInput")
    bk = nc.dram_tensor("bk", [FL], F32, kind="ExternalInput")
    bob = nc.dram_tensor("bob", [128, C], F32, kind="ExternalInput")  # (bo/2+bv@Wo) bcast
    ident = nc.dram_tensor("ident", [128, 128], F32R, kind="ExternalInput")
    identb = nc.dram_tensor("identb", [128, 128], BF16, kind="ExternalInput")
    mask4 = nc.dram_tensor("mask4", [128, 128], F32, kind="ExternalInput")
    zh = nc.dram_tensor("zh", [T // 2, C], F32, kind="ExternalOutput")

    with tile.TileContext(nc) as tc:
        with (
            tc.tile_pool(name="res", bufs=1) as res,
            tc.tile_pool(name="dram", bufs=1, space="DRAM") as dram,
        ):
            # resident: Q^T, K^T bf16 [128, 4hp x 2048tok]; V+ones bf16
            # [128, 16tb x 520]; mask, identity + out-proj consts.
            qt_sb = res.tile([128, NHP * T], BF16)
            kt_sb = res.tile([128, NHP * T], BF16)
            v_sb = res.tile([128, NTB * (NHL * 65)], BF16)
            m4_sb = res.tile([128, 128], F32, name="m4")
            wo_sb = res.tile([128, NHP * C], BF16, name="wo_sb")
            bob_sb = res.tile([128, C], F32, name="bob_sb")
            id_sb = res.tile([128, 128], F32R, name="id_sb")
            idb_sb = res.tile([128, 128], BF16, name="idb_sb")
            zpart = dram.tile([T, C], F32)
            zreds = [
                dram.tile([128, C], F32, name=f"zred{i}") for i in range(8)
            ]

            with (
                tc.tile_pool(name="p1c", bufs=1) as p1c,
                tc.tile_pool(name="p1", bufs=6) as p1,
                tc.tile_pool(name="xtp", bufs=1) as xtp,
                tc.tile_pool(name="ytp", bufs=4) as ytp,
                tc.tile_pool(name="ysb_pool", bufs=1) as ysb_pool,
                tc.tile_pool(name="p2", bufs=10) as p2,
                tc.tile_pool(name="norm", bufs=8) as norm,
                tc.tile_pool(name="p3", bufs=6) as p3,
                tc.tile_pool(name="s_ps", bufs=2, space="PSUM") as s_ps_pool,
                tc.tile_pool(name="yu_ps", bufs=2, space="PSUM") as yu_ps_pool,
                tc.tile_pool(name="scr_ps", bufs=2, space="PSUM") as scr_ps_pool,
            ):
                ysb = ysb_pool.tile([128, NHP * T], BF16)

                # ---- preamble DMAs (order = SP queue order) ----
                xnats = {}

                def load_xnat(tb):
                    xnats[tb] = p1.tile([128, C], BF16, tag="xnat",
                                        name=f"xnat{tb}")
                    nc.sync.dma_start(xnats[tb][:], xb[tb * 128:(tb + 1) * 128, :])

                load_xnat(0)
                nc.sync.dma_start(idb_sb[:], identb[:, :])
                nc.sync.dma_start(id_sb[:], ident[:, :])
                for tb in range(1, 4):
                    load_xnat(tb)
                # K/Q weights in per-fb chunks, in first-use order, so the
                # first K matmul group can start as soon as possible
                wk_sb = p1c.tile([128, NCC * FL], BF16, tag="wk")
                wq_sb = p1c.tile([128, NCC * FL], BF16, tag="wq")
                wv_sb = p1c.tile([128, NCC * FL], BF16, tag="wv")

                for w_sb, w_dram in ((wk_sb, wk), (wq_sb, wq)):
                    nc.sync.dma_start(
                        w_sb[:].rearrange("p (c f) -> p c f", c=NCC),
                        w_dram.rearrange("(c p) f -> p c f", p=128),
                    )
                bq_sb = p1c.tile([128, NHP], F32, tag="bq")
                nc.sync.dma_start(bq_sb[:], bq.rearrange("(f p) -> p f", p=128))
                bk_sb = p1c.tile([128, NHP], F32, tag="bk")
                nc.sync.dma_start(bk_sb[:], bk.rearrange("(f p) -> p f", p=128))
                # warm the exp table (hides ~2.7us ACT_TABLE_LOAD)
                warm = p1c.tile([1, 1], F32, tag="warm")
                nc.scalar.activation(warm[:], id_sb[0:1, 0:1].bitcast(F32), AF.Exp)
                for half in range(2):
                    nc.sync.dma_start(
                        wv_sb[:, half * 4 * FL:(half + 1) * 4 * FL].rearrange(
                            "p (c f) -> p c f", c=NCC // 2),
                        wv[half * 512:(half + 1) * 512, :].rearrange(
                            "(c p) f -> p c f", p=128),
                    )
                for tb in range(4, 8):
                    load_xnat(tb)
                nc.sync.dma_start(m4_sb[:], mask4[:, :])
                nc.sync.dma_start(
                    wo_sb[:].rearrange("p (c n) -> p c n", c=NHP),
                    wo.rearrange("(c p) n -> p c n", p=128),
                )
                nc.sync.dma_start(bob_sb[:], bob[:, :])

                # ---- phase-1 pieces: x^T transposes and QKV projections ----
                xts = {}

                def xt_alloc(tt):
                    xts[tt] = xtp.tile([128, NCC * 512], BF16, tag="xt",
                                       name=f"xt{tt}")

                def tp_piece(tt, k):
                    # transpose token block tb = 4*tt+k into x^T tile tt
                    # (blocks 0-7 are queued in the preamble)
                    tb = 4 * tt + k
                    if tb + 4 < NTB and tb + 4 >= 8:
                        load_xnat(tb + 4)
                    xnat = xnats.pop(tb)
                    xt = xts[tt]
                    for cg in range(NCC // 4):
                        tp_ps = scr_ps_pool.tile([128, 512], BF16, tag="scr",
                                                 name=f"tp{tb}_{cg}")
                        for kk in range(4):
                            cc = cg * 4 + kk
                            nc.tensor.transpose(
                                tp_ps[:, kk * 128:(kk + 1) * 128],
                                xnat[:, cc * 128:(cc + 1) * 128], idb_sb[:]
                            )
                        # alternate DVE/Act so the copy chain isn't serialized
                        dst = xt[:].rearrange("p (c t) -> p c t", c=NCC)[
                            :, cg * 4:(cg + 1) * 4, k * 128:(k + 1) * 128
                        ]
                        src = tp_ps[:].rearrange("p (k t) -> p k t", k=4)
                        nc.vector.tensor_scalar_add(dst, src, 0.0)

                def qk_piece(tt, fb, w_sb, b_sb, dst):
                    xt = xts[tt]
                    ps = scr_ps_pool.tile([128, 512], F32, tag="scr",
                                          name=f"ps{tt}_{fb}")
                    for cc in range(NCC):
                        nc.tensor.matmul(
                            ps[:],
                            w_sb[:, cc * FL + fb * 128: cc * FL + (fb + 1) * 128],
                            xt[:, cc * 512:(cc + 1) * 512],
                            start=(cc == 0),
                            stop=(cc == NCC - 1),
                        )
                    nc.vector.tensor_scalar_add(
                        dst[:, fb * T + tt * 512: fb * T + (tt + 1) * 512],
                        ps[:],
                        b_sb[:, fb:fb + 1],
                    )

                def v_piece(tt, k):
                    tb = 4 * tt + k
                    xt = xts[tt]
                    ps = scr_ps_pool.tile([128, 512], F32, tag="scr",
                                          name=f"psv{tb}")
                    for cc in range(NCC):
                        nc.tensor.matmul(
                            ps[:],
                            xt[:, cc * 512 + k * 128: cc * 512 + (k + 1) * 128],
                            wv_sb[:, cc * FL:(cc + 1) * FL],
                            start=(cc == 0),
                            stop=(cc == NCC - 1),
                        )
                    vslice = v_sb[:, tb * (NHL * 65):(tb + 1) * (NHL * 65)]
                    v3 = vslice.rearrange("p (h c) -> p h c", h=NHL)
                    nc.vector.tensor_scalar_add(
                        v3[:, :, 0:D],
                        ps[:].rearrange("p (h d) -> p h d", h=NHL),
                        0.0,
                    )
                    nc.gpsimd.memset(v3[:, :, D:D + 1], 1.0)

                def qkv_pieces(tt):
                    return (
                        [lambda fb=fb: qk_piece(tt, fb, wk_sb, bk_sb, kt_sb)
                         for fb in range(NHP)]
                        + [lambda fb=fb: qk_piece(tt, fb, wq_sb, bq_sb, qt_sb)
                           for fb in range(NHP)]
                        + [lambda k=k: v_piece(tt, k) for k in range(4)]
                    )

                def tp_pieces(tt):
                    def first(k):
                        def go():
                            if k == 0:
                                xt_alloc(tt)
                            tp_piece(tt, k)
                        return go
                    return [first(k) for k in range(4)]

                # ---- out-proj + ReduceScatter pieces ----
                # zpart rows chunk-major: chunk c holds tb c then tb 8+c, so
                # each pairwise ReduceScatter chunk is one contiguous block.
                ZROW = {}
                for c in range(8):
                    ZROW[c] = c * 256
                    ZROW[8 + c] = c * 256 + 128

                def op_chunk(tb, ct):
                    zrow = ZROW[tb]
                    zps = scr_ps_pool.tile([128, 512], F32, tag="scr",
                                           name=f"z{tb}_{ct}")
                    for cc in range(NHP):
                        nc.tensor.matmul(
                            zps[:],
                            ysb[:, cc * T + tb * 128: cc * T + (tb + 1) * 128],
                            wo_sb[:, cc * C + ct * 512: cc * C + (ct + 1) * 512],
                            start=(cc == 0),
                            stop=(cc == NHP - 1),
                        )
                    z_sb = p3.tile(
                        [128, 512], F32, tag="zsb", name=f"zsb{tb}_{ct}"
                    )
                    nc.vector.tensor_add(
                        z_sb[:], zps[:], bob_sb[:, ct * 512:(ct + 1) * 512]
                    )
                    nc.sync.dma_start(
                        zpart[zrow:zrow + 128, ct * 512:(ct + 1) * 512],
                        z_sb[:],
                    )

                def rs_chunk(c):
                    nc.gpsimd.collective_compute(
                        "ReduceScatter",
                        mybir.AluOpType.add,
                        replica_groups=[[0, 1], [2, 3], [4, 5], [6, 7]],
                        ins=[zpart[c * 256:(c + 1) * 256, :].opt()],
                        outs=[zreds[c].opt()],
                    )
                    nc.sync.dma_start(
                        zh[c * 128:(c + 1) * 128, :], zreds[c][:]
                    )

                def op_item(tb, ct, c=None):
                    def go():
                        op_chunk(tb, ct)
                        if c is not None:
                            rs_chunk(c)
                    return go

                # ---- piece queue: dripped into attention units ----
                queue = []
                emitted = [0]

                def drip():
                    if queue:
                        emitted[0] += 1
                        queue.pop(0)()

                def drain(n_left=0):
                    while len(queue) > n_left:
                        drip()

                # ---- attention for one (qtile, head-pair) unit ----
                y_ts = {}

                def attention_unit(qt, hp):
                    if qt not in y_ts:
                        # y_t: qtile's y, token-major [128 tok, 4tb x 512f]
                        y_ts[qt] = ytp.tile([128, 4 * 512], BF16, tag="yt",
                                            name=f"yt{qt}")
                    y_t = y_ts[qt]
                    if True:
                        n_kb = 4 * (qt + 1)
                        n_kg = n_kb // 2
                        # yu_t: attention out transposed, [128 q, 4qoff x
                        # (64d + rowsum)] per hi; queries on partitions so the
                        # softmax denominator is a per-partition scalar.
                        yus = [
                            yu_ps_pool.tile([128, 4 * 65], F32, tag="yu",
                                            name=f"yu{qt}_{hp}_{i}")
                            for i in range(2)
                        ]
                        qsl = qt_sb[:, hp * T + qt * 512: hp * T + (qt + 1) * 512]
                        sss = {}

                        def emit_scores(kg, hi=None):
                            if hi is None:
                                sss[kg] = [
                                    s_ps_pool.tile([128, 1024], F32, tag="s",
                                                   name=f"s{qt}_{hp}_{kg}_{i}")
                                    for i in range(2)
                                ]
                                for h2 in range(2):
                                    emit_scores(kg, h2)
                                return
                            ss = sss[kg]
                            for c2 in range(2):
                                kb = kg * 2 + c2
                                c = kb - 4 * qt
                                # c==1 writes the full block so the
                                # untrimmed exp never reads unwritten
                                # PSUM (the extra cols are discarded)
                                j0 = c * 128 if c >= 2 else 0
                                nc.tensor.matmul(
                                    ss[hi][:, c2 * 512 + j0:(c2 + 1) * 512],
                                    kt_sb[
                                        hi * 64:(hi + 1) * 64,
                                        hp * T + kb * 128:
                                        hp * T + (kb + 1) * 128,
                                    ],
                                    qsl[hi * 64:(hi + 1) * 64, j0:],
                                    tile_position=(hi * 64, 0),
                                    start=True,
                                    stop=True,
                                )
                            for c2 in range(2):
                                c = kg * 2 + c2 - 4 * qt
                                if 0 <= c <= 3:
                                    # triangular band [c*128,(c+1)*128):
                                    # same i>j triangle for every c
                                    b0 = c2 * 512 + c * 128
                                    nc.vector.tensor_add(
                                        ss[hi][:, b0:b0 + 128],
                                        ss[hi][:, b0:b0 + 128],
                                        m4_sb[:, 0:128],
                                    )

                        # software pipeline: scores(kg+1) emitted BEFORE
                        # attnV(kg) so the exp chain never starves the Act.
                        emit_scores(0)
                        for kg in range(n_kg):
                            ss = sss.pop(kg)
                            j0r = []
                            for c2 in range(2):
                                c = kg * 2 + c2 - 4 * qt
                                j0r.append(c * 128 if c > 0 else 0)
                            # per-hi: exp -> scores(kg+1,hi) -> attnV(kg,hi)
                            # so the PE serves work in readiness order (the
                            # hi1 scores are only legal after exp(kg,hi1) and
                            # must not block the already-ready attnV(kg,hi0))
                            for hi in range(2):
                                at = p2.tile([128, 1024], BF16, tag="attn")
                                if j0r[0] >= 256:
                                    # heavily masked pair: exp valid suffixes
                                    nc.scalar.activation(
                                        at[:, j0r[0]:512], ss[hi][:, j0r[0]:512],
                                        AF.Exp, scale=0.125,
                                    )
                                    nc.scalar.activation(
                                        at[:, 512 + j0r[1]:1024],
                                        ss[hi][:, 512 + j0r[1]:1024],
                                        AF.Exp, scale=0.125,
                                    )
                                else:
                                    nc.scalar.activation(
                                        at[:], ss[hi][:], AF.Exp, scale=0.125
                                    )
                                if kg + 1 < n_kg:
                                    if hi == 0:
                                        sss[kg + 1] = [
                                            s_ps_pool.tile(
                                                [128, 1024], F32, tag="s",
                                                name=f"s{qt}_{hp}_{kg + 1}_{i}")
                                            for i in range(2)
                                        ]
                                    emit_scores(kg + 1, hi)
                                h = 2 * hp + hi
                                for c2 in range(2):
                                    kb = kg * 2 + c2
                                    vsl = v_sb[
                                        :,
                                        kb * (NHL * 65) + h * 65:
                                        kb * (NHL * 65) + h * 65 + 65,
                                    ]
                                    # one accumulation group per yu PSUM bank:
                                    # start=True clears the whole 2KB zero
                                    # region, so only the very first write may
                                    # set it; later first-touch columns are
                                    # overwritten via per-element has_written.
                                    for qoff in range(max(kb - 4 * qt, 0), 4):
                                        nc.tensor.matmul(
                                            yus[hi][:, qoff * 65:
                                                   (qoff + 1) * 65],
                                            at[:, c2 * 512 + qoff * 128:
                                               c2 * 512 + (qoff + 1) * 128],
                                            vsl,
                                            start=(kb == 0 and qoff == 0),
                                            stop=(kb == n_kb - 1),
                                        )
                            drip()
                        # normalize: y = y_u * (1/rowsum); rowsum is col 64 of
                        # each 65-block = a per-partition scalar. No broadcast.
                        for hi in range(2):
                            yu3 = yus[hi][:].rearrange("p (b c) -> p b c", b=4)
                            rcp = norm.tile([128, 4], F32, tag="rcp")
                            nc.vector.reciprocal(
                                rcp[:].unsqueeze(2), yu3[:, :, 64:65]
                            )
                            h = 2 * hp + hi
                            for qoff in range(4):
                                nc.vector.tensor_scalar_mul(
                                    y_t[:, qoff * 512 + h * 64:
                                        qoff * 512 + h * 64 + 64],
                                    yus[hi][:, qoff * 65: qoff * 65 + 64],
                                    rcp[:, qoff:qoff + 1],
                                )
                def finish_qt(qt):
                    # transpose y_t -> ysb feature-major for the out-proj
                    y_t = y_ts[qt]
                    for tl in range(4):
                        tb = 4 * qt + tl
                        tp = scr_ps_pool.tile([128, 512], BF16, tag="scr",
                                              name=f"ytp{qt}_{tl}")
                        for fc in range(NHP):
                            nc.tensor.transpose(
                                tp[:, fc * 128:(fc + 1) * 128],
                                y_t[:, tl * 512 + fc * 128:
                                    tl * 512 + (fc + 1) * 128],
                                idb_sb[:],
                            )
                        nc.vector.tensor_scalar_add(
                            ysb[:].rearrange("p (h t) -> p h t", h=NHP)[
                                :, :, tb * 128:(tb + 1) * 128],
                            tp[:].rearrange("p (h t) -> p h t", h=NHP),
                            0.0,
                        )
                    # hold all out-proj chunks for the last qtile's
                    # window, where the PE otherwise starves (Act-bound)
                    if qt == 2:
                        queue.extend(op_item(tb, ct)
                                     for tb in range(8) for ct in range(2))
                        queue.extend(op_item(8 + c, ct, c if ct else None)
                                     for c in range(4) for ct in range(2))

                # ---- schedule ----
                # prologue: QKV of tile 0 and transposes of tiles 0-1 run
                # before any attention (PE-heavy, Act idle).
                for piece in tp_pieces(0):
                    piece()
                for piece in qkv_pieces(0):
                    piece()
                for piece in tp_pieces(1):
                    piece()

                # queue pieces in dependency order, recording for each
                # attention unit (qt,hp) how many pieces must be emitted
                # before it (its K/Q columns and all V of its key range).
                prereq = {(0, hp): 0 for hp in range(NHP)}
                total = [0]

                def app(ps):
                    queue.extend(ps)
                    total[0] += len(ps)

                def app_qkv(tt):
                    qp = qkv_pieces(tt)  # [K fb0..3, Q fb0..3, V k0..3]
                    app([qp[0], qp[4]])          # K fb0, Q fb0
                    app(qp[8:12])                # V k0..3
                    prereq[(tt, 0)] = total[0]
                    for fb in range(1, NHP):
                        app([qp[fb], qp[4 + fb]])
                        prereq[(tt, fb)] = total[0]

                app_qkv(1)
                app(tp_pieces(2))
                app_qkv(2)
                app(tp_pieces(3))
                app_qkv(3)

                unit_seq = [(0, 0), (0, 1), (0, 2), (0, 3),
                            (1, 0), (1, 1), (1, 2), (1, 3),
                            (2, 0), (2, 1), (2, 2), (2, 3),
                            (3, 0), (3, 1), (3, 2), (3, 3)]
                left = {qt: NHP for qt in range(NTT)}
                for qt, hp in unit_seq:
                    while emitted[0] < prereq[(qt, hp)] and queue:
                        drip()
                    attention_unit(qt, hp)
                    left[qt] -= 1
                    if left[qt] == 0:
                        finish_qt(qt)
                drain()
                for i in range(4):
                    op_chunk(12 + i, 0)
                    op_chunk(12 + i, 1)
                    rs_chunk(4 + i)

    nc.compile()
    return nc


_NC_CACHE = None


def _get_nc():
    global _NC_CACHE
    if _NC_CACHE is None:
        _NC_CACHE = build()
    return _NC_CACHE


def _in_maps(x, Wqkv, bqkv, Wo, bo):
    x = np.ascontiguousarray(np.asarray(x, dtype=np.float32))
    Wqkv = np.ascontiguousarray(np.asarray(Wqkv, dtype=np.float32))
    bqkv = np.asarray(bqkv, dtype=np.float32)
    Wo = np.ascontiguousarray(np.asarray(Wo, dtype=np.float32))
    bo = np.asarray(bo, dtype=np.float32)

    ident = np.eye(128, dtype=np.float32)
    i_ = np.arange(128, dtype=np.int64)[:, None]
    j_ = np.arange(128, dtype=np.int64)[None, :]
    mask4 = np.where(i_ > j_, np.float32(NEG), np.float32(0.0)).astype(np.float32)

    from ml_dtypes import bfloat16
    identb = np.eye(128, dtype=bfloat16)
    in_maps = []
    for core in range(8):
        b, hh = core // 2, core % 2
        sl = slice(hh * FL, (hh + 1) * FL)
        bv_loc = bqkv[2 * C:][sl]
        wo_loc = np.ascontiguousarray(Wo[sl, :])
        # V bias folded into output bias: attn rows sum to 1 after normalize
        bo_loc = bo * 0.5 + bv_loc @ wo_loc
        in_maps.append({
            "xb": x[b].astype(bfloat16),
            "wq": np.ascontiguousarray(Wqkv[:, 0 * C:1 * C][:, sl]).astype(bfloat16),
            "wk": np.ascontiguousarray(Wqkv[:, 1 * C:2 * C][:, sl]).astype(bfloat16),
            "wv": np.ascontiguousarray(Wqkv[:, 2 * C:3 * C][:, sl]).astype(bfloat16),
            "wo": wo_loc.astype(bfloat16),
            "bq": np.ascontiguousarray(bqkv[0 * C:1 * C][sl]),
            "bk": np.ascontiguousarray(bqkv[1 * C:2 * C][sl]),
            "bob": np.broadcast_to(bo_loc[None, :], (128, C)).copy(),
            "ident": ident,
            "identb": identb,
            "mask4": mask4,
        })

    return in_maps


def _assemble(res):
    out = np.empty((B, T, C), dtype=np.float32)
    for b in range(B):
        out[b, : T // 2] = res.results[2 * b]["zh"]
        out[b, T // 2:] = res.results[2 * b + 1]["zh"]
    return out


def kernel(x, Wqkv, bqkv, Wo, bo):
    in_maps = _in_maps(x, Wqkv, bqkv, Wo, bo)
    res = run_bass_kernel_spmd(_get_nc(), in_maps, core_ids=list(range(8)))
    return _assemble(res)


def run_traced(x, Wqkv, bqkv, Wo, bo, trace_cores=None):
    in_maps = _in_maps(x, Wqkv, bqkv, Wo, bo)
    res = run_bass_kernel_spmd(
        _get_nc(), in_maps, core_ids=list(range(8)), trace=True,
        trace_cores=trace_cores,
    )
    return res


# revision 84
# speedup vs baseline: 1.0062x; 1.0004x over previous
"""Causal multi-head attention block (QKV proj -> causal MHA -> out proj) on 8 Trainium2
cores.

Sharding: core = b*2 + hh handles batch b (of 4) and head-half hh (8 of 16 heads),
computing attention for its heads over the full sequence, then a partial output
projection over its 512 y-channels for all 2048 tokens. A pairwise ReduceScatter
([0,1],[2,3],...) sums the two partials of each batch and leaves each core with its
token-half of the final output. Host-side work is pure slicing/concatenation.

The QKV projection (per 512-token tile) is interleaved with the attention of the
previous query tile so the PE fills the Act-bound softmax phase: emission alternates
attention inner-loop iterations with "pieces" (QKV matmul groups, x^T transposes,
out-proj chunks). Attention itself is software-pipelined: scores(kg+1) is emitted
before attnV(kg) so the exp chain on the Act engine never starves.

attnV computes y transposed (queries on PSUM partitions, via a ones column in V for
the softmax denominator) so normalization is a per-partition tensor_scalar multiply;
y is then transposed back on the PE for the output projection.

dtypes: bf16 throughout the matmul operands (x^T, Q^T/K^T/V, attention weights,
y, Wo — inputs converted host-side); all PSUM accumulation is f32 and the softmax
denominator is accumulated from the same bf16 weights, so normalization is exact
w.r.t. rounding. The V bias is folded into the output bias host-side (attn rows
sum to 1 after normalization).
"""

import numpy as np

import concourse.bass as bass
import concourse.tile as tile
from concourse import bacc, mybir
from concourse.bass_utils import run_bass_kernel_spmd

F32 = mybir.dt.float32
F32R = mybir.dt.float32r
BF16 = mybir.dt.bfloat16
AF = mybir.ActivationFunctionType

B, T, C, H = 4, 2048, 1024, 16
D = C // H          # 64
NHL = H // 2        # 8 local heads per core
NHP = NHL // 2      # 4 local head pairs
FL = NHL * D        # 512 local features
NCC = C // 128      # 8 contraction chunks over C
NTB = T // 128      # 16 token blocks
NTT = T // 512      # 4 token tiles / qtiles
NEG = -1.0e30


def build():
    nc = bacc.Bacc("TRN2", target_bir_lowering=False, num_devices=8)

    xb = nc.dram_tensor("xb", [T, C], BF16, kind="ExternalInput")
    wq = nc.dram_tensor("wq", [C, FL], BF16, kind="ExternalInput")
    wk = nc.dram_tensor("wk", [C, FL], BF16, kind="ExternalInput")
    wv = nc.dram_tensor("wv", [C, FL], BF16, kind="ExternalInput")
    wo = nc.dram_tensor("wo", [FL, C], BF16, kind="ExternalInput")
    bq = nc.dram_tensor("bq", [FL], F32, kind="ExternalInput")
    bk = nc.dram_tensor("bk", [FL], F32, kind="ExternalInput")
    bob = nc.dram_tensor("bob", [128, C], F32, kind="ExternalInput")  # (bo/2+bv@Wo) bcast
    ident = nc.dram_tensor("ident", [128, 128], F32R, kind="ExternalInput")
    identb = nc.dram_tensor("identb", [128, 128], BF16, kind="ExternalInput")
    mask4 = nc.dram_tensor("mask4", [128, 128], F32, kind="ExternalInput")
    zh = nc.dram_tensor("zh", [T // 2, C], F32, kind="ExternalOutput")

    with tile.TileContext(nc) as tc:
        with (
            tc.tile_pool(name="res", bufs=1) as res,
            tc.tile_pool(name="dram", bufs=1, space="DRAM") as dram,
        ):
            # resident: Q^T, K^T bf16 [128, 4hp x 2048tok]; V+ones bf16
            # [128, 16tb x 520]; mask, identity + out-proj consts.
            qt_sb = res.tile([128, NHP * T], BF16)
            kt_sb = res.tile([128, NHP * T], BF16)
            v_sb = res.tile([128, NTB * (NHL * 65)], BF16)
            m4_sb = res.tile([128, 128], F32, name="m4")
            wo_sb = res.tile([128, NHP * C], BF16, name="wo_sb")
            bob_sb = res.tile([128, C], F32, name="bob_sb")
            id_sb = res.tile([128, 128], F32R, name="id_sb")
            idb_sb = res.tile([128, 128], BF16, name="idb_sb")
            zpart = dram.tile([T, C], F32)
            zreds = [
                dram.tile([128, C], F32, name=f"zred{i}") for i in range(8)
            ]

            with (
                tc.tile_pool(name="p1c", bufs=1) as p1c,
                tc.tile_pool(name="p1", bufs=6) as p1,
                tc.tile_pool(name="xtp", bufs=1) as xtp,
                tc.tile_pool(name="ytp", bufs=4) as ytp,
                tc.tile_pool(name="ysb_pool", bufs=1) as ysb_pool,
                tc.tile_pool(name="p2", bufs=10) as p2,
                tc.tile_pool(name="norm", bufs=8) as norm,
                tc.tile_pool(name="p3", bufs=6) as p3,
                tc.tile_pool(name="s_ps", bufs=2, space="PSUM") as s_ps_pool,
                tc.tile_pool(name="yu_ps", bufs=2, space="PSUM") as yu_ps_pool,
                tc.tile_pool(name="scr_ps", bufs=2, space="PSUM") as scr_ps_pool,
            ):
                ysb = ysb_pool.tile([128, NHP * T], BF16)

                # ---- preamble DMAs (order = SP queue order) ----
                xnats = {}

                def load_xnat(tb):
                    xnats[tb] = p1.tile([128, C], BF16, tag="xnat",
                                        name=f"xnat{tb}")
                    nc.sync.dma_start(xnats[tb][:], xb[tb * 128:(tb + 1) * 128, :])

                load_xnat(0)
                nc.sync.dma_start(idb_sb[:], identb[:, :])
                nc.sync.dma_start(id_sb[:], ident[:, :])
                for tb in range(1, 4):
                    load_xnat(tb)
                # K/Q weights in per-fb chunks, in first-use order, so the
                # first K matmul group can start as soon as possible
                wk_sb = p1c.tile([128, NCC * FL], BF16, tag="wk")
                wq_sb = p1c.tile([128, NCC * FL], BF16, tag="wq")
                wv_sb = p1c.tile([128, NCC * FL], BF16, tag="wv")

                # halves: the cc-accumulation loop can start on the
                # first four chunks while the second half is in flight
                for w_sb, w_dram in ((wk_sb, wk), (wq_sb, wq)):
                    for half in range(2):
                        nc.sync.dma_start(
                            w_sb[:, half * 4 * FL:(half + 1) * 4 * FL]
                            .rearrange("p (c f) -> p c f", c=NCC // 2),
                            w_dram[half * 512:(half + 1) * 512, :].rearrange(
                                "(c p) f -> p c f", p=128),
                        )
                bq_sb = p1c.tile([128, NHP], F32, tag="bq")
                nc.sync.dma_start(bq_sb[:], bq.rearrange("(f p) -> p f", p=128))
                bk_sb = p1c.tile([128, NHP], F32, tag="bk")
                nc.sync.dma_start(bk_sb[:], bk.rearrange("(f p) -> p f", p=128))
                # warm the exp table (hides ~2.7us ACT_TABLE_LOAD)
                warm = p1c.tile([1, 1], F32, tag="warm")
                nc.scalar.activation(warm[:], id_sb[0:1, 0:1].bitcast(F32), AF.Exp)
                for half in range(2):
                    nc.sync.dma_start(
                        wv_sb[:, half * 4 * FL:(half + 1) * 4 * FL].rearrange(
                            "p (c f) -> p c f", c=NCC // 2),
                        wv[half * 512:(half + 1) * 512, :].rearrange(
                            "(c p) f -> p c f", p=128),
                    )
                for tb in range(4, 8):
                    load_xnat(tb)
                nc.sync.dma_start(m4_sb[:], mask4[:, :])
                nc.sync.dma_start(
                    wo_sb[:].rearrange("p (c n) -> p c n", c=NHP),
                    wo.rearrange("(c p) n -> p c n", p=128),
                )
                nc.sync.dma_start(bob_sb[:], bob[:, :])

                # ---- phase-1 pieces: x^T transposes and QKV projections ----
                xts = {}

                def xt_alloc(tt):
                    xts[tt] = xtp.tile([128, NCC * 512], BF16, tag="xt",
                                       name=f"xt{tt}")

                def tp_piece(tt, k):
                    # transpose token block tb = 4*tt+k into x^T tile tt
                    # (blocks 0-7 are queued in the preamble)
                    tb = 4 * tt + k
                    if tb + 4 < NTB and tb + 4 >= 8:
                        load_xnat(tb + 4)
                    xnat = xnats.pop(tb)
                    xt = xts[tt]
                    for cg in range(NCC // 4):
                        tp_ps = scr_ps_pool.tile([128, 512], BF16, tag="scr",
                                                 name=f"tp{tb}_{cg}")
                        for kk in range(4):
                            cc = cg * 4 + kk
                            nc.tensor.transpose(
                                tp_ps[:, kk * 128:(kk + 1) * 128],
                                xnat[:, cc * 128:(cc + 1) * 128], idb_sb[:]
                            )
                        # alternate DVE/Act so the copy chain isn't serialized
                        dst = xt[:].rearrange("p (c t) -> p c t", c=NCC)[
                            :, cg * 4:(cg + 1) * 4, k * 128:(k + 1) * 128
                        ]
                        src = tp_ps[:].rearrange("p (k t) -> p k t", k=4)
                        nc.vector.tensor_scalar_add(dst, src, 0.0)

                def qk_piece(tt, fb, w_sb, b_sb, dst):
                    xt = xts[tt]
                    ps = scr_ps_pool.tile([128, 512], F32, tag="scr",
                                          name=f"ps{tt}_{fb}")
                    for cc in range(NCC):
                        nc.tensor.matmul(
                            ps[:],
                            w_sb[:, cc * FL + fb * 128: cc * FL + (fb + 1) * 128],
                            xt[:, cc * 512:(cc + 1) * 512],
                            start=(cc == 0),
                            stop=(cc == NCC - 1),
                        )
                    nc.vector.tensor_scalar_add(
                        dst[:, fb * T + tt * 512: fb * T + (tt + 1) * 512],
                        ps[:],
                        b_sb[:, fb:fb + 1],
                    )

                def v_piece(tt, k):
                    tb = 4 * tt + k
                    xt = xts[tt]
                    ps = scr_ps_pool.tile([128, 512], F32, tag="scr",
                                          name=f"psv{tb}")
                    for cc in range(NCC):
                        nc.tensor.matmul(
                            ps[:],
                            xt[:, cc * 512 + k * 128: cc * 512 + (k + 1) * 128],
                            wv_sb[:, cc * FL:(cc + 1) * FL],
                            start=(cc == 0),
                            stop=(cc == NCC - 1),
                        )
                    vslice = v_sb[:, tb * (NHL * 65):(tb + 1) * (NHL * 65)]
                    v3 = vslice.rearrange("p (h c) -> p h c", h=NHL)
                    nc.vector.tensor_scalar_add(
                        v3[:, :, 0:D],
                        ps[:].rearrange("p (h d) -> p h d", h=NHL),
                        0.0,
                    )
                    nc.gpsimd.memset(v3[:, :, D:D + 1], 1.0)

                def qkv_pieces(tt):
                    return (
                        [lambda fb=fb: qk_piece(tt, fb, wk_sb, bk_sb, kt_sb)
                         for fb in range(NHP)]
                        + [lambda fb=fb: qk_piece(tt, fb, wq_sb, bq_sb, qt_sb)
                           for fb in range(NHP)]
                        + [lambda k=k: v_piece(tt, k) for k in range(4)]
                    )

                def tp_pieces(tt):
                    def first(k):
                        def go():
                            if k == 0:
                                xt_alloc(tt)
                            tp_piece(tt, k)
                        return go
                    return [first(k) for k in range(4)]

                # ---- out-proj + ReduceScatter pieces ----
                # zpart rows chunk-major: chunk c holds tb c then tb 8+c, so
                # each pairwise ReduceScatter chunk is one contiguous block.
                ZROW = {}
                for c in range(8):
                    ZROW[c] = c * 256
                    ZROW[8 + c] = c * 256 + 128

                def op_chunk(tb, ct):
                    zrow = ZROW[tb]
                    zps = scr_ps_pool.tile([128, 512], F32, tag="scr",
                                           name=f"z{tb}_{ct}")
                    for cc in range(NHP):
                        nc.tensor.matmul(
                            zps[:],
                            ysb[:, cc * T + tb * 128: cc * T + (tb + 1) * 128],
                            wo_sb[:, cc * C + ct * 512: cc * C + (ct + 1) * 512],
                            start=(cc == 0),
                            stop=(cc == NHP - 1),
                        )
                    z_sb = p3.tile(
                        [128, 512], F32, tag="zsb", name=f"zsb{tb}_{ct}"
                    )
                    nc.vector.tensor_add(
                        z_sb[:], zps[:], bob_sb[:, ct * 512:(ct + 1) * 512]
                    )
                    nc.sync.dma_start(
                        zpart[zrow:zrow + 128, ct * 512:(ct + 1) * 512],
                        z_sb[:],
                    )

                def rs_chunk(c):
                    nc.gpsimd.collective_compute(
                        "ReduceScatter",
                        mybir.AluOpType.add,
                        replica_groups=[[0, 1], [2, 3], [4, 5], [6, 7]],
                        ins=[zpart[c * 256:(c + 1) * 256, :].opt()],
                        outs=[zreds[c].opt()],
                    )
                    nc.sync.dma_start(
                        zh[c * 128:(c + 1) * 128, :], zreds[c][:]
                    )

                def op_item(tb, ct, c=None):
                    def go():
                        op_chunk(tb, ct)
                        if c is not None:
                            rs_chunk(c)
                    return go

                # ---- piece queue: dripped into attention units ----
                queue = []
                emitted = [0]

                def drip():
                    if queue:
                        emitted[0] += 1
                        queue.pop(0)()

                def drain(n_left=0):
                    while len(queue) > n_left:
                        drip()

                # ---- attention for one (qtile, head-pair) unit ----
                y_ts = {}

                def attention_unit(qt, hp):
                    if qt not in y_ts:
                        # y_t: qtile's y, token-major [128 tok, 4tb x 512f]
                        y_ts[qt] = ytp.tile([128, 4 * 512], BF16, tag="yt",
                                            name=f"yt{qt}")
                    y_t = y_ts[qt]
                    if True:
                        n_kb = 4 * (qt + 1)
                        n_kg = n_kb // 2
                        # yu_t: attention out transposed, [128 q, 4qoff x
                        # (64d + rowsum)] per hi; queries on partitions so the
                        # softmax denominator is a per-partition scalar.
                        yus = [
                            yu_ps_pool.tile([128, 4 * 65], F32, tag="yu",
                                            name=f"yu{qt}_{hp}_{i}")
                            for i in range(2)
                        ]
                        qsl = qt_sb[:, hp * T + qt * 512: hp * T + (qt + 1) * 512]
                        sss = {}

                        def emit_scores(kg, hi=None):
                            if hi is None:
                                sss[kg] = [
                                    s_ps_pool.tile([128, 1024], F32, tag="s",
                                                   name=f"s{qt}_{hp}_{kg}_{i}")
                                    for i in range(2)
                                ]
                                for h2 in range(2):
                                    emit_scores(kg, h2)
                                return
                            ss = sss[kg]
                            for c2 in range(2):
                                kb = kg * 2 + c2
                                c = kb - 4 * qt
                                # c==1 writes the full block so the
                                # untrimmed exp never reads unwritten
                                # PSUM (the extra cols are discarded)
                                j0 = c * 128 if c >= 2 else 0
                                nc.tensor.matmul(
                                    ss[hi][:, c2 * 512 + j0:(c2 + 1) * 512],
                                    kt_sb[
                                        hi * 64:(hi + 1) * 64,
                                        hp * T + kb * 128:
                                        hp * T + (kb + 1) * 128,
                                    ],
                                    qsl[hi * 64:(hi + 1) * 64, j0:],
                                    tile_position=(hi * 64, 0),
                                    start=True,
                                    stop=True,
                                )
                            for c2 in range(2):
                                c = kg * 2 + c2 - 4 * qt
                                if 0 <= c <= 3:
                                    # triangular band [c*128,(c+1)*128):
                                    # same i>j triangle for every c
                                    b0 = c2 * 512 + c * 128
                                    nc.vector.tensor_add(
                                        ss[hi][:, b0:b0 + 128],
                                        ss[hi][:, b0:b0 + 128],
                                        m4_sb[:, 0:128],
                                    )

                        # software pipeline: scores(kg+1) emitted BEFORE
                        # attnV(kg) so the exp chain never starves the Act.
                        emit_scores(0)
                        for kg in range(n_kg):
                            ss = sss.pop(kg)
                            j0r = []
                            for c2 in range(2):
                                c = kg * 2 + c2 - 4 * qt
                                j0r.append(c * 128 if c > 0 else 0)
                            # per-hi: exp -> scores(kg+1,hi) -> attnV(kg,hi)
                            # so the PE serves work in readiness order (the
                            # hi1 scores are only legal after exp(kg,hi1) and
                            # must not block the already-ready attnV(kg,hi0))
                            for hi in range(2):
                                at = p2.tile([128, 1024], BF16, tag="attn")
                                if j0r[0] >= 256:
                                    # heavily masked pair: exp valid suffixes
                                    nc.scalar.activation(
                                        at[:, j0r[0]:512], ss[hi][:, j0r[0]:512],
                                        AF.Exp, scale=0.125,
                                    )
                                    nc.scalar.activation(
                                        at[:, 512 + j0r[1]:1024],
                                        ss[hi][:, 512 + j0r[1]:1024],
                                        AF.Exp, scale=0.125,
                                    )
                                else:
                                    nc.scalar.activation(
                                        at[:], ss[hi][:], AF.Exp, scale=0.125
                                    )
                                if kg + 1 < n_kg:
                                    if hi == 0:
                                        sss[kg + 1] = [
                                            s_ps_pool.tile(
                                                [128, 1024], F32, tag="s",
                                                name=f"s{qt}_{hp}_{kg + 1}_{i}")
                                            for i in range(2)
                                        ]
                                    emit_scores(kg + 1, hi)
                                h = 2 * hp + hi
                                for c2 in range(2):
                                    kb = kg * 2 + c2
                                    vsl = v_sb[
                                        :,
                                        kb * (NHL * 65) + h * 65:
                                        kb * (NHL * 65) + h * 65 + 65,
                                    ]
                                    # one accumulation group per yu PSUM bank:
                                    # start=True clears the whole 2KB zero
                                    # region, so only the very first write may
                                    # set it; later first-touch columns are
                                    # overwritten via per-element has_written.
                                    for qoff in range(max(kb - 4 * qt, 0), 4):
                                        nc.tensor.matmul(
                                            yus[hi][:, qoff * 65:
                                                   (qoff + 1) * 65],
                                            at[:, c2 * 512 + qoff * 128:
                                               c2 * 512 + (qoff + 1) * 128],
                                            vsl,
                                            start=(kb == 0 and qoff == 0),
                                            stop=(kb == n_kb - 1),
                                        )
                            drip()
                        # normalize: y = y_u * (1/rowsum); rowsum is col 64 of
                        # each 65-block = a per-partition scalar. No broadcast.
                        for hi in range(2):
                            yu3 = yus[hi][:].rearrange("p (b c) -> p b c", b=4)
                            rcp = norm.tile([128, 4], F32, tag="rcp")
                            nc.vector.reciprocal(
                                rcp[:].unsqueeze(2), yu3[:, :, 64:65]
                            )
                            h = 2 * hp + hi
                            for qoff in range(4):
                                nc.vector.tensor_scalar_mul(
                                    y_t[:, qoff * 512 + h * 64:
                                        qoff * 512 + h * 64 + 64],
                                    yus[hi][:, qoff * 65: qoff * 65 + 64],
                                    rcp[:, qoff:qoff + 1],
                                )
                def finish_qt(qt):
                    # transpose y_t -> ysb feature-major for the out-proj
                    y_t = y_ts[qt]
                    for tl in range(4):
                        tb = 4 * qt + tl
                        tp = scr_ps_pool.tile([128, 512], BF16, tag="scr",
                                              name=f"ytp{qt}_{tl}")
                        for fc in range(NHP):
                            nc.tensor.transpose(
                                tp[:, fc * 128:(fc + 1) * 128],
                                y_t[:, tl * 512 + fc * 128:
                                    tl * 512 + (fc + 1) * 128],
                                idb_sb[:],
                            )
                        nc.vector.tensor_scalar_add(
                            ysb[:].rearrange("p (h t) -> p h t", h=NHP)[
                                :, :, tb * 128:(tb + 1) * 128],
                            tp[:].rearrange("p (h t) -> p h t", h=NHP),
                            0.0,
                        )
                    # hold all out-proj chunks for the last qtile's
                    # window, where the PE otherwise starves (Act-bound)
                    if qt == 2:
                        queue.extend(op_item(tb, ct)
                                     for tb in range(8) for ct in range(2))
                        queue.extend(op_item(8 + c, ct, c if ct else None)
                                     for c in range(4) for ct in range(2))

                # ---- schedule ----
                # prologue: QKV of tile 0 and transposes of tiles 0-1 run
                # before any attention (PE-heavy, Act idle).
                for piece in tp_pieces(0):
                    piece()
                for piece in qkv_pieces(0):
                    piece()
                for piece in tp_pieces(1):
                    piece()

                # queue pieces in dependency order, recording for each
                # attention unit (qt,hp) how many pieces must be emitted
                # before it (its K/Q columns and all V of its key range).
                prereq = {(0, hp): 0 for hp in range(NHP)}
                total = [0]

                def app(ps):
                    queue.extend(ps)
                    total[0] += len(ps)

                def app_qkv(tt):
                    qp = qkv_pieces(tt)  # [K fb0..3, Q fb0..3, V k0..3]
                    app([qp[0], qp[4]])          # K fb0, Q fb0
                    app(qp[8:12])                # V k0..3
                    prereq[(tt, 0)] = total[0]
                    for fb in range(1, NHP):
                        app([qp[fb], qp[4 + fb]])
                        prereq[(tt, fb)] = total[0]

                app_qkv(1)
                app(tp_pieces(2))
                app_qkv(2)
                app(tp_pieces(3))
                app_qkv(3)

                unit_seq = [(0, 0), (0, 1), (0, 2), (0, 3),
                            (1, 0), (1, 1), (1, 2), (1, 3),
                            (2, 0), (2, 1), (2, 2), (2, 3),
                            (3, 0), (3, 1), (3, 2), (3, 3)]
                left = {qt: NHP for qt in range(NTT)}
                for qt, hp in unit_seq:
                    while emitted[0] < prereq[(qt, hp)] and queue:
                        drip()
                    attention_unit(qt, hp)
                    left[qt] -= 1
                    if left[qt] == 0:
                        finish_qt(qt)
                drain()
                for i in range(4):
                    op_chunk(12 + i, 0)
                    op_chunk(12 + i, 1)
                    rs_chunk(4 + i)

    nc.compile()
    return nc


_NC_CACHE = None


def _get_nc():
    global _NC_CACHE
    if _NC_CACHE is None:
        _NC_CACHE = build()
    return _NC_CACHE


def _in_maps(x, Wqkv, bqkv, Wo, bo):
    x = np.ascontiguousarray(np.asarray(x, dtype=np.float32))
    Wqkv = np.ascontiguousarray(np.asarray(Wqkv, dtype=np.float32))
    bqkv = np.asarray(bqkv, dtype=np.float32)
    Wo = np.ascontiguousarray(np.asarray(Wo, dtype=np.float32))
    bo = np.asarray(bo, dtype=np.float32)

    ident = np.eye(128, dtype=np.float32)
    i_ = np.arange(128, dtype=np.int64)[:, None]
    j_ = np.arange(128, dtype=np.int64)[None, :]
    mask4 = np.where(i_ > j_, np.float32(NEG), np.float32(0.0)).astype(np.float32)

    from ml_dtypes import bfloat16
    identb = np.eye(128, dtype=bfloat16)
    in_maps = []
    for core in range(8):
        b, hh = core // 2, core % 2
        sl = slice(hh * FL, (hh + 1) * FL)
        bv_loc = bqkv[2 * C:][sl]
        wo_loc = np.ascontiguousarray(Wo[sl, :])
        # V bias folded into output bias: attn rows sum to 1 after normalize
        bo_loc = bo * 0.5 + bv_loc @ wo_loc
        in_maps.append({
            "xb": x[b].astype(bfloat16),
            "wq": np.ascontiguousarray(Wqkv[:, 0 * C:1 * C][:, sl]).astype(bfloat16),
            "wk": np.ascontiguousarray(Wqkv[:, 1 * C:2 * C][:, sl]).astype(bfloat16),
            "wv": np.ascontiguousarray(Wqkv[:, 2 * C:3 * C][:, sl]).astype(bfloat16),
            "wo": wo_loc.astype(bfloat16),
            "bq": np.ascontiguousarray(bqkv[0 * C:1 * C][sl]),
            "bk": np.ascontiguousarray(bqkv[1 * C:2 * C][sl]),
            "bob": np.broadcast_to(bo_loc[None, :], (128, C)).copy(),
            "ident": ident,
            "identb": identb,
            "mask4": mask4,
        })

    return in_maps


def _assemble(res):
    out = np.empty((B, T, C), dtype=np.float32)
    for b in range(B):
        out[b, : T // 2] = res.results[2 * b]["zh"]
        out[b, T // 2:] = res.results[2 * b + 1]["zh"]
    return out


def kernel(x, Wqkv, bqkv, Wo, bo):
    in_maps = _in_maps(x, Wqkv, bqkv, Wo, bo)
    res = run_bass_kernel_spmd(_get_nc(), in_maps, core_ids=list(range(8)))
    return _assemble(res)


def run_traced(x, Wqkv, bqkv, Wo, bo, trace_cores=None):
    in_maps = _in_maps(x, Wqkv, bqkv, Wo, bo)
    res = run_bass_kernel_spmd(
        _get_nc(), in_maps, core_ids=list(range(8)), trace=True,
        trace_cores=trace_cores,
    )
    return res
